# revision 6
# baseline (speedup 1.0000x reference)
"""Causal GQA self-attention with RoPE on 8 Trainium2 NeuronCores.

Sharding: tensor-parallel over heads. Each core owns 2 q-heads and their
(shared) kv-head: it projects q/k/v for all 4096 tokens, applies RoPE, runs
causal attention, then an AllToAll redistributes attention outputs so each
core o-projects a 512-token slice with the full Wo. Host assembles slices.

All matmuls run in bf16 with fp32 PSUM accumulation. The RoPE interleaved
pair rotation is turned into a contiguous half rotation by permuting the
rows of Wq/Wk on the host (even head-dims first); q.k dot products are
invariant under applying the same permutation to q and k.

Shapes (hardcoded from the problem spec):
  x [2, 2048, 2048] f32, Wq [2048, 2048], Wk/Wv [512, 2048], Wo [2048, 2048],
  position_ids [2048] i32.  16 q-heads, 4 kv-heads, head_dim 128.
"""

from contextlib import ExitStack

import ml_dtypes
import numpy as np

import concourse.bass as bass
import concourse.tile as tile
from concourse import bacc, mybir
from concourse.bass_utils import run_bass_kernel_spmd
from concourse.masks import make_identity

B, S, E = 2, 2048, 2048
H, HKV, D = 16, 4, 128
NCORES = 8
HPC = H // NCORES          # q-heads per core
T = B * S                  # 4096 flattened tokens
TSL = T // NCORES          # 512-token o_proj slice per core
NE = E // 128              # 16 contraction chunks
ROPE_THETA = 10000.0
SCALE = 1.0 / float(np.sqrt(D))

BF16 = mybir.dt.bfloat16
FP32 = mybir.dt.float32
AF = mybir.ActivationFunctionType

_cached_nc = None


def _build_nc():
    nc = bacc.Bacc(None, target_bir_lowering=False, debug=False, num_devices=NCORES)

    xT = nc.dram_tensor("xT", [E, T], BF16, kind="ExternalInput")
    wqkvT = nc.dram_tensor("wqkvT", [E, 512], BF16, kind="ExternalInput")
    woT = nc.dram_tensor("woT", [E, E], BF16, kind="ExternalInput")
    cosT = nc.dram_tensor("cosT", [64, T], BF16, kind="ExternalInput")
    sinT = nc.dram_tensor("sinT", [64, T], BF16, kind="ExternalInput")
    masks = nc.dram_tensor("masks", [128, 4 * 512], BF16, kind="ExternalInput")
    outT = nc.dram_tensor("outT", [E, TSL], FP32, kind="ExternalOutput")

    a2a_in = nc.dram_tensor("a2a_in", [NCORES * HPC * D, TSL], BF16)
    a2a_out = nc.dram_tensor("a2a_out", [NCORES * HPC * D, TSL], BF16)

    with tile.TileContext(nc) as tc, ExitStack() as ctx:
        # ---- persistent SBUF ----
        const_pool = ctx.enter_context(tc.tile_pool(name="const", bufs=1))
        qkv_pool = ctx.enter_context(tc.tile_pool(name="qkv", bufs=1))

        cos_sb = const_pool.tile([64, T], BF16, tag="cos")
        sin_sb = const_pool.tile([64, T], BF16, tag="sin")
        mask_sb = const_pool.tile([128, 4 * 512], BF16, tag="mask")
        nc.sync.dma_start(cos_sb[:], cosT[:, :])
        nc.sync.dma_start(sin_sb[:], sinT[:, :])
        nc.sync.dma_start(mask_sb[:], masks[:, :])

        ident = const_pool.tile([128, 128], FP32, tag="ident")
        make_identity(nc, ident[:])
        ones_col = const_pool.tile([128, 1], FP32, tag="ones_col")
        nc.gpsimd.memset(ones_col[:], 1.0)
        ones_row = const_pool.tile([1, 128], FP32, tag="ones_row")
        nc.gpsimd.memset(ones_row[:], 1.0)

        # q [128, HPC*T] (head-major cols), k [128, T], v stored transposed as
        # 128-token column blocks: v_sb[:, c*128:(c+1)*128] = v[tokens c][:, d]
        q_sb = qkv_pool.tile([128, HPC * T], BF16, tag="q")
        k_sb = qkv_pool.tile([128, T], BF16, tag="k")
        v_sb = qkv_pool.tile([128, T], BF16, tag="v")

        w_pool = ctx.enter_context(tc.tile_pool(name="w", bufs=NE))
        w_sb = []
        for e in range(NE):
            wt = w_pool.tile([128, 512], BF16, tag="wqkv")
            nc.sync.dma_start(wt[:], wqkvT[e * 128:(e + 1) * 128, :])
            w_sb.append(wt)

        # ---- phase 1: qkv projection + rope + v transpose ----
        phase1 = ExitStack()
        x_pool = phase1.enter_context(tc.tile_pool(name="x", bufs=18))
        proj_psum = phase1.enter_context(tc.tile_pool(name="pproj", bufs=3, space="PSUM"))
        tr_psum = phase1.enter_context(tc.tile_pool(name="ptr", bufs=2, space="PSUM"))
        rope_tmp = phase1.enter_context(tc.tile_pool(name="ropetmp", bufs=8))
        v_tmp = phase1.enter_context(tc.tile_pool(name="vtmp", bufs=2))

        QT = 1024  # tokens per quarter
        for tq in range(T // QT):
            xq = []
            for e in range(NE):
                xt = x_pool.tile([128, QT], BF16, tag="x")
                nc.sync.dma_start(
                    xt[:], xT[e * 128:(e + 1) * 128, tq * QT:(tq + 1) * QT])
                xq.append(xt)
            for dt in range(4):  # q-head0, q-head1, k, v
                for half in range(2):
                    ps = proj_psum.tile([128, 512], FP32, tag="proj")
                    for e in range(NE):
                        nc.tensor.matmul(
                            ps[:],
                            lhsT=w_sb[e][:, dt * 128:(dt + 1) * 128],
                            rhs=xq[e][:, half * 512:(half + 1) * 512],
                            start=(e == 0),
                            stop=(e == NE - 1),
                        )
                    col = tq * QT + half * 512  # global token offset
                    if dt < 3:
                        # rope: rows 0:64 = even head dims, 64:128 = odd
                        dst = q_sb if dt < 2 else k_sb
                        dcol = dt * T + col if dt < 2 else col
                        c_sl = cos_sb[:, col:col + 512]
                        s_sl = sin_sb[:, col:col + 512]
                        t1 = rope_tmp.tile([64, 512], FP32, tag="rt")
                        t2 = rope_tmp.tile([64, 512], FP32, tag="rt")
                        t3 = rope_tmp.tile([64, 512], FP32, tag="rt")
                        t4 = rope_tmp.tile([64, 512], FP32, tag="rt")
                        top, bot = ps[0:64, :], ps[64:128, :]
                        nc.vector.tensor_mul(t1[:], top, c_sl)
                        nc.vector.tensor_mul(t2[:], bot, s_sl)
                        nc.vector.tensor_sub(dst[0:64, dcol:dcol + 512], t1[:], t2[:])
                        nc.vector.tensor_mul(t3[:], bot, c_sl)
                        nc.vector.tensor_mul(t4[:], top, s_sl)
                        nc.vector.tensor_add(dst[64:128, dcol:dcol + 512], t3[:], t4[:])
                    else:
                        # v: cast then PE-transpose each 128-token block
                        vt = v_tmp.tile([128, 512], FP32, tag="vt")
                        nc.scalar.copy(vt[:], ps[:])
                        for j in range(4):
                            pt = tr_psum.tile([128, 128], FP32, tag="tr")
                            nc.tensor.transpose(
                                pt[:], vt[:, j * 128:(j + 1) * 128], ident[:])
                            cblk = col + j * 128
                            nc.scalar.copy(v_sb[:, cblk:cblk + 128], pt[:])

        phase1.close()

        # ---- phase 2: causal attention per (batch, head) ----
        phase2 = ExitStack()
        s_psum = phase2.enter_context(tc.tile_pool(name="ps", bufs=2, space="PSUM"))
        y_psum = phase2.enter_context(tc.tile_pool(name="py", bufs=2, space="PSUM"))
        d_psum = phase2.enter_context(tc.tile_pool(name="pd", bufs=1, space="PSUM"))
        b_psum = phase2.enter_context(tc.tile_pool(name="pb", bufs=1, space="PSUM"))
        e_pool = phase2.enter_context(tc.tile_pool(name="e", bufs=6))
        rs_pool = phase2.enter_context(tc.tile_pool(name="rs", bufs=2))
        sm_pool = phase2.enter_context(tc.tile_pool(name="sm", bufs=4))

        for b in range(B):
            for h in range(HPC):
                q_base = h * T + b * S
                for qc in range(S // 512):
                    q0 = qc * 512
                    nkt = (q0 + 512) // 128
                    ps_y = y_psum.tile([128, 512], FP32, tag="y")
                    rs = rs_pool.tile([128, 512], FP32, tag="rs")
                    for kt in range(nkt):
                        ps_s = s_psum.tile([128, 512], FP32, tag="s")
                        nc.tensor.matmul(
                            ps_s[:],
                            lhsT=k_sb[:, b * S + kt * 128:b * S + (kt + 1) * 128],
                            rhs=q_sb[:, q_base + q0:q_base + q0 + 512],
                            start=True, stop=True,
                        )
                        e_t = e_pool.tile([128, 512], BF16, tag="e")
                        nc.scalar.activation(e_t[:], ps_s[:], AF.Exp, scale=SCALE)
                        r = kt * 128 - q0
                        if r >= 0:
                            ri = r // 128
                            nc.vector.tensor_mul(
                                e_t[:], e_t[:], mask_sb[:, ri * 512:(ri + 1) * 512])
                        nc.tensor.matmul(
                            ps_y[:],
                            lhsT=v_sb[:, (b * 16 + kt) * 128:(b * 16 + kt + 1) * 128],
                            rhs=e_t[:],
                            start=(kt == 0), stop=(kt == nkt - 1),
                        )
                        if kt == 0:
                            nc.vector.tensor_copy(rs[:], e_t[:])
                        else:
                            nc.vector.tensor_add(rs[:], rs[:], e_t[:])
                    # softmax denominator -> reciprocal -> broadcast -> scale
                    ps_d = d_psum.tile([1, 512], FP32, tag="d")
                    nc.tensor.matmul(ps_d[:], lhsT=ones_col[:], rhs=rs[:],
                                     start=True, stop=True)
                    recip = sm_pool.tile([1, 512], FP32, tag="recip")
                    nc.vector.reciprocal(recip[:], ps_d[:])
                    ps_b = b_psum.tile([128, 512], FP32, tag="bc")
                    nc.tensor.matmul(ps_b[:], lhsT=ones_row[:], rhs=recip[:],
                                     start=True, stop=True)
                    rb = sm_pool.tile([128, 512], FP32, tag="rb")
                    nc.scalar.copy(rb[:], ps_b[:])
                    y_t = sm_pool.tile([128, 512], BF16, tag="yt")
                    nc.vector.tensor_mul(y_t[:], ps_y[:], rb[:])
                    # a2a_in shard j=(4b+qc): rows j*256 + h*128
                    j = 4 * b + qc
                    row = j * HPC * D + h * 128
                    nc.sync.dma_start(a2a_in[row:row + 128, :], y_t[:])

        # ---- all-to-all: heads -> token slices ----
        nc.gpsimd.collective_compute(
            "AllToAll",
            mybir.AluOpType.bypass,
            replica_groups=[list(range(NCORES))],
            ins=[a2a_in[:, :]],
            outs=[a2a_out[:, :]],
        )

        phase2.close()

        # ---- phase 3: o_proj for this core's 512-token slice ----
        yag_pool = ctx.enter_context(tc.tile_pool(name="yag", bufs=NE))
        wo_pool = ctx.enter_context(tc.tile_pool(name="wo", bufs=34))
        o_psum = ctx.enter_context(tc.tile_pool(name="po", bufs=2, space="PSUM"))
        o_pool = ctx.enter_context(tc.tile_pool(name="osb", bufs=2))

        yag = []
        for yd in range(NE):
            yt = yag_pool.tile([128, TSL], BF16, tag="yag")
            nc.sync.dma_start(yt[:], a2a_out[yd * 128:(yd + 1) * 128, :])
            yag.append(yt)
        for ot in range(NE):
            ps_o = o_psum.tile([128, TSL], FP32, tag="o")
            for yd in range(NE):
                wt = wo_pool.tile([128, 128], BF16, tag="wo")
                nc.sync.dma_start(
                    wt[:], woT[yd * 128:(yd + 1) * 128, ot * 128:(ot + 1) * 128])
                nc.tensor.matmul(ps_o[:], lhsT=wt[:], rhs=yag[yd][:],
                                 start=(yd == 0), stop=(yd == NE - 1))
            o_sb = o_pool.tile([128, TSL], FP32, tag="osb")
            nc.scalar.copy(o_sb[:], ps_o[:])
            nc.sync.dma_start(outT[ot * 128:(ot + 1) * 128, :], o_sb[:])

    nc.compile()
    return nc


def _prep_inputs(x, Wq, Wk, Wv, Wo, position_ids):
    bf16 = ml_dtypes.bfloat16
    xT = np.ascontiguousarray(
        x.reshape(T, E).T).astype(bf16)

    # rope permutation: even head-dims first, then odd
    perm = np.concatenate([np.arange(0, D, 2), np.arange(1, D, 2)])
    Wq_p = Wq.reshape(H, D, E)[:, perm, :]
    Wk_p = Wk.reshape(HKV, D, E)[:, perm, :]
    Wv_r = Wv.reshape(HKV, D, E)

    pos = position_ids.astype(np.float64)
    inv_freq = 1.0 / (ROPE_THETA ** (np.arange(0, D, 2, dtype=np.float64) / D))
    freqs = pos[:, None] * inv_freq[None, :]            # [S, 64]
    cosT = np.tile(np.cos(freqs).T, (1, B)).astype(bf16)  # [64, T] batch-major
    sinT = np.tile(np.sin(freqs).T, (1, B)).astype(bf16)

    # diagonal-band causal masks: mask[r][k, q] = 1 if k + 128*r <= q
    kk = np.arange(128)[:, None]
    qq = np.arange(512)[None, :]
    m = np.zeros((128, 4, 512), np.float32)
    for r in range(4):
        m[:, r, :] = (kk + 128 * r <= qq).astype(np.float32)
    masks = np.ascontiguousarray(m.reshape(128, 4 * 512)).astype(bf16)

    woT = np.ascontiguousarray(Wo.T).astype(bf16)       # [yd, o]

    in_maps = []
    for c in range(NCORES):
        wq_c = Wq_p[2 * c:2 * c + 2].reshape(HPC * D, E)     # [256, E]
        g = c // 2
        wqkvT = np.concatenate(
            [wq_c.T, Wk_p[g].T, Wv_r[g].T], axis=1).astype(bf16)  # [E, 512]
        in_maps.append({
            "xT": xT,
            "wqkvT": np.ascontiguousarray(wqkvT),
            "woT": woT,
            "cosT": cosT,
            "sinT": sinT,
            "masks": masks,
        })
    return in_maps


def kernel(x, Wq, Wk, Wv, Wo, position_ids):
    global _cached_nc
    if _cached_nc is None:
        _cached_nc = _build_nc()
    nc = _cached_nc

    in_maps = _prep_inputs(
        np.asarray(x, np.float32), np.asarray(Wq, np.float32),
        np.asarray(Wk, np.float32), np.asarray(Wv, np.float32),
        np.asarray(Wo, np.float32), np.asarray(position_ids))

    res = run_bass_kernel_spmd(nc, in_maps, core_ids=list(range(NCORES)))

    out = np.concatenate(
        [res.results[c]["outT"].T for c in range(NCORES)], axis=0)
    return np.ascontiguousarray(out.reshape(B, S, E).astype(np.float32))
